# revision 5
# baseline (speedup 1.0000x reference)
"""Trainium2 Bass kernel for nn_MultiHeadAttention (B=2, S=2048, D=1024, H=16, dk=dv=64).

Sharding: 8 cores = 2 batches x 4 head-groups (4 heads each).

Device-side architecture (per core), all in "transposed" orientation so that
every big matmul contracts on the partition dim and every big DMA moves long
contiguous per-partition lines:

  Launch A (attention):
    qhT/khT = Wq^T-chunks (stationary) @ qT/kT  -> [64, S] per head (bf16)
    vh      = vT-chunks (stationary) @ Wv       -> [S, 64] per head (+ ones col)
    per (head, k-chunk):
      S^T psum[k,q] =  I_fp8 @ maskbiasT  (+)  khT_chunk^T @ qhT   (PE)
      E^T = Exp(S^T)                             (ACT, psum->sbuf bf16)
      [outT | sums] += [vh | ones]^T @ E^T       (PE, accumulated over k)
    recip = Exp(-Log(sums)); broadcast via DRAM bounce
    attn^T chunk = E^T * recip   -> DRAM (host transposes per-head 2D slabs)
    outT_norm = outT * recip -> DRAM (bf16)
  Launch B (fc + residual + layernorm) on 512-row slices after host re-shard:
    fc psum = outTg-chunks (stationary) @ Wfc; x = fc + residual
    mean/var via bn_stats; out = (x-mu)*rstd*gamma + beta

The harness calls kernel(**inputs) with full inputs; host code here only
shards / transposes / casts / gathers (no math beyond layout).
"""

import numpy as np
import ml_dtypes
from contextlib import ExitStack

import concourse.bass as bass
import concourse.bacc as bacc
import concourse.tile as tile
from concourse import mybir
from concourse.bass_utils import run_bass_kernel_spmd

F32 = mybir.dt.float32
BF16 = mybir.dt.bfloat16
FP8 = mybir.dt.float8e5
AF = mybir.ActivationFunctionType

N_HEAD, D_MODEL, D_K, D_V = 16, 1024, 64, 64
B, S = 2, 2048
HG = 4              # heads per core (head group)
N_CORES = 8
EPS = 1e-6
MASK_NEG = -57344.0  # max-magnitude finite float8_e5m2; exp(x-57344) == 0

TRACE = False       # set True by test.py for a profiled run


# --------------------------------------------------------------------------
# Launch A: attention
# --------------------------------------------------------------------------

def build_kernel_a():
    nc = bacc.Bacc("TRN2", target_bir_lowering=False, debug=False,
                   num_devices=N_CORES)
    qT = nc.dram_tensor("qT", [D_MODEL, S], BF16, kind="ExternalInput").ap()
    kT = nc.dram_tensor("kT", [D_MODEL, S], BF16, kind="ExternalInput").ap()
    vT = nc.dram_tensor("vT", [D_MODEL, S], BF16, kind="ExternalInput").ap()
    mbT = nc.dram_tensor("maskbT", [S, S], FP8, kind="ExternalInput").ap()
    wq = nc.dram_tensor("wq", [D_MODEL, HG * D_K], BF16, kind="ExternalInput").ap()
    wk = nc.dram_tensor("wk", [D_MODEL, HG * D_K], BF16, kind="ExternalInput").ap()
    wv = nc.dram_tensor("wv", [D_MODEL, HG * D_V], BF16, kind="ExternalInput").ap()
    ident = nc.dram_tensor("ident", [128, 128], FP8, kind="ExternalInput").ap()
    attnT = nc.dram_tensor("attnT", [HG, S, S], F32, kind="ExternalOutput").ap()
    outT = nc.dram_tensor("outT", [HG, D_V, S], BF16, kind="ExternalOutput").ap()

    NKC = S // 128        # 16 k-chunks
    NCC = D_MODEL // 128  # 8 contraction chunks

    with tile.TileContext(nc) as tc, ExitStack() as ctx:
        const = ctx.enter_context(tc.tile_pool(name="const", bufs=1))
        ident_sb = const.tile([128, 128], FP8)
        nc.sync.dma_start(out=ident_sb, in_=ident)
        wq_sb = const.tile([128, NCC, HG * D_K], BF16)
        wk_sb = const.tile([128, NCC, HG * D_K], BF16)
        wv_sb = const.tile([128, NCC, HG * D_V], BF16)
        nc.sync.dma_start(out=wq_sb, in_=wq.rearrange("(a p) m -> p a m", p=128))
        nc.sync.dma_start(out=wk_sb, in_=wk.rearrange("(a p) m -> p a m", p=128))
        nc.sync.dma_start(out=wv_sb, in_=wv.rearrange("(a p) m -> p a m", p=128))

        # persistent per-head projections
        heads = ctx.enter_context(tc.tile_pool(name="heads", bufs=1))
        qhT = [heads.tile([D_K, S], BF16, tag=f"qhT{j}", name=f"qhT{j}") for j in range(HG)]
        khT = [heads.tile([D_K, S], BF16, tag=f"khT{j}", name=f"khT{j}") for j in range(HG)]
        # vh: per k-chunk [128, HG*(D_V+1)] = [v_h | ones] blocks
        vh = [heads.tile([128, HG * (D_V + 1)], BF16, tag=f"vh{kc}", name=f"vh{kc}")
              for kc in range(NKC)]
        outT_sb = [heads.tile([D_V, S], BF16, tag=f"ot{j}", name=f"ot{j}") for j in range(HG)]

        # ---------------- projections ----------------
        with tc.tile_pool(name="pin", bufs=1) as pin, \
             tc.tile_pool(name="ppsum", bufs=3, space="PSUM") as ppsum:
            for name, src, w_sb, dsts in (
                ("q", qT, wq_sb, qhT), ("k", kT, wk_sb, khT)):
                src_sb = pin.tile([128, NCC, S], BF16, tag="pin", name="pin")
                nc.sync.dma_start(out=src_sb,
                                  in_=src.rearrange("(a p) s -> p a s", p=128))
                for hp in range(HG // 2):          # head pairs
                    for sc in range(S // 512):
                        ps = ppsum.tile([128, 512], F32, tag="ps", name="ps")
                        for cc in range(NCC):
                            nc.tensor.matmul(
                                ps,
                                lhsT=w_sb[:, cc, hp * 128:(hp + 1) * 128],
                                rhs=src_sb[:, cc, sc * 512:(sc + 1) * 512],
                                start=(cc == 0), stop=(cc == NCC - 1))
                        sl = slice(sc * 512, (sc + 1) * 512)
                        nc.scalar.copy(out=dsts[2 * hp][:, sl], in_=ps[0:D_K, :])
                        nc.vector.tensor_copy(out=dsts[2 * hp + 1][:, sl],
                                              in_=ps[D_K:128, :])
            # v projection (normal orientation) + ones columns
            src_sb = pin.tile([128, NCC, S], BF16, tag="pin", name="pin")
            nc.sync.dma_start(out=src_sb,
                              in_=vT.rearrange("(a p) s -> p a s", p=128))
            for kc in range(NKC):
                ps = ppsum.tile([128, 512], F32, tag="ps", name="ps")
                for cc in range(NCC):
                    nc.tensor.matmul(
                        ps[:, 0:HG * D_V],
                        lhsT=src_sb[:, cc, kc * 128:(kc + 1) * 128],
                        rhs=wv_sb[:, cc, :],
                        start=(cc == 0), stop=(cc == NCC - 1))
                vt = vh[kc].rearrange("p (h x) -> p h x", h=HG)
                nc.vector.tensor_copy(
                    out=vt[:, :, 0:D_V],
                    in_=ps[:, 0:HG * D_V].rearrange("p (h d) -> p h d", h=HG))
                nc.vector.memset(vt[:, :, D_V:D_V + 1], 1.0)

        # ---------------- attention pairs ----------------
        mbp = ctx.enter_context(tc.tile_pool(name="mbp", bufs=2))
        epool = ctx.enter_context(tc.tile_pool(name="epool", bufs=NKC))
        spsum = ctx.enter_context(tc.tile_pool(name="spsum", bufs=2, space="PSUM"))
        avpsum = ctx.enter_context(tc.tile_pool(name="avpsum", bufs=1, space="PSUM"))
        npool = ctx.enter_context(tc.tile_pool(name="npool", bufs=2))
        rbpool = ctx.enter_context(tc.tile_pool(name="rbpool", bufs=1))
        aopool = ctx.enter_context(tc.tile_pool(name="aopool", bufs=2))
        dpool = ctx.enter_context(tc.tile_pool(name="dpool", bufs=2, space="DRAM"))

        for j in range(HG):
            avp = avpsum.tile([D_V + 1, S], F32, tag="avp", name="avp")
            ets = []
            for kc in range(NKC):
                mb = mbp.tile([128, S], FP8, tag="mb", name="mb")
                nc.sync.dma_start(out=mb, in_=mbT[kc * 128:(kc + 1) * 128, :])
                et = epool.tile([128, S], BF16, tag="et", name="et")
                ets.append(et)
                sps = [spsum.tile([128, 1024], F32, tag="sp", name="sp") for _ in range(2)]
                for u in range(4):
                    sl = slice(u * 512, (u + 1) * 512)
                    nc.tensor.matmul(sps[u // 2][:, (u % 2) * 512:(u % 2 + 1) * 512],
                                     lhsT=ident_sb, rhs=mb[:, sl],
                                     start=True, stop=False)
                for u in range(4):
                    sl = slice(u * 512, (u + 1) * 512)
                    nc.tensor.matmul(sps[u // 2][:, (u % 2) * 512:(u % 2 + 1) * 512],
                                     lhsT=khT[j][:, kc * 128:(kc + 1) * 128],
                                     rhs=qhT[j][:, sl],
                                     start=False, stop=True)
                for t in range(2):
                    nc.scalar.activation(out=et[:, t * 1024:(t + 1) * 1024],
                                         in_=sps[t], func=AF.Exp)
                for u in range(4):
                    sl = slice(u * 512, (u + 1) * 512)
                    nc.tensor.matmul(
                        avp[:, sl],
                        lhsT=vh[kc][:, j * (D_V + 1):(j + 1) * (D_V + 1)],
                        rhs=et[:, sl],
                        start=(kc == 0), stop=(kc == NKC - 1))

            # normalization factors: recip = exp(-ln(sums)), bcast via DRAM
            nc.scalar.activation(out=avp[D_V:D_V + 1, :], in_=avp[D_V:D_V + 1, :],
                                 func=AF.Ln)
            rec = npool.tile([1, S], F32, tag="rec", name="rec")
            nc.scalar.activation(out=rec, in_=avp[D_V:D_V + 1, :],
                                 func=AF.Exp, scale=-1.0)
            rec_d = dpool.tile([1, S], F32, tag="rd", name="rd")
            nc.sync.dma_start(out=rec_d, in_=rec)
            recb = npool.tile([128, S], F32, tag="recb", name="recb")
            bcast = bass.AP(tensor=rec_d.tensor, offset=rec_d.offset,
                            ap=[[0, 128]] + [list(d) for d in rec_d.ap])
            nc.gpsimd.dma_start(out=recb, in_=bcast)
            recb16 = rbpool.tile([128, S], BF16, tag="recb16", name="recb16")
            nc.vector.tensor_copy(out=recb16, in_=recb)

            # attn-V output (normalized), bf16
            nc.vector.tensor_mul(out=outT_sb[j], in0=avp[0:D_V, :],
                                 in1=recb[0:D_V, :])
            nc.sync.dma_start(out=outT[j], in_=outT_sb[j])

            # attn output (normalized), fp32, [k, q] orientation
            for kc in range(NKC):
                ao = aopool.tile([128, S], F32, tag="ao", name="ao")
                nc.vector.tensor_mul(out=ao, in0=ets[kc], in1=recb16)
                nc.sync.dma_start(out=attnT[j, kc * 128:(kc + 1) * 128, :],
                                  in_=ao)

    nc.compile()
    return nc


# --------------------------------------------------------------------------
# Launch B: fc + residual + layernorm on a 512-row slice
# --------------------------------------------------------------------------

def build_kernel_b():
    nc = bacc.Bacc("TRN2", target_bir_lowering=False, debug=False,
                   num_devices=N_CORES)
    SL = S // 4  # 512 rows per core
    og = nc.dram_tensor("outTg", [N_HEAD * D_V, SL], BF16,
                        kind="ExternalInput").ap()
    wfc = nc.dram_tensor("wfc", [N_HEAD * D_V, D_MODEL], BF16,
                         kind="ExternalInput").ap()
    resid = nc.dram_tensor("resid", [SL, D_MODEL], F32,
                           kind="ExternalInput").ap()
    gr = nc.dram_tensor("gamma_rep", [128, D_MODEL], F32,
                        kind="ExternalInput").ap()
    br = nc.dram_tensor("beta_rep", [128, D_MODEL], F32,
                        kind="ExternalInput").ap()
    outp = nc.dram_tensor("outp", [SL, D_MODEL], F32, kind="ExternalOutput").ap()

    NHC = (N_HEAD * D_V) // 128  # 8

    with tile.TileContext(nc) as tc, ExitStack() as ctx:
        const = ctx.enter_context(tc.tile_pool(name="const", bufs=1))
        og_sb = const.tile([128, NHC, SL], BF16)
        nc.sync.dma_start(out=og_sb, in_=og.rearrange("(a p) s -> p a s", p=128))
        wf_sb = const.tile([128, NHC, D_MODEL], BF16)
        nc.sync.dma_start(out=wf_sb, in_=wfc.rearrange("(a p) m -> p a m", p=128))
        gr_sb = const.tile([128, D_MODEL], F32)
        nc.sync.dma_start(out=gr_sb, in_=gr)
        br_sb = const.tile([128, D_MODEL], F32)
        nc.sync.dma_start(out=br_sb, in_=br)
        eps_sb = const.tile([128, 1], F32)
        nc.vector.memset(eps_sb, EPS)

        work = ctx.enter_context(tc.tile_pool(name="work", bufs=3))
        stp = ctx.enter_context(tc.tile_pool(name="stp", bufs=4))
        fpsum = ctx.enter_context(tc.tile_pool(name="fpsum", bufs=2, space="PSUM"))

        for qc in range(SL // 128):
            ps = fpsum.tile([128, D_MODEL], F32, tag="ps", name="ps")
            for hc in range(NHC):
                for mc in range(2):
                    nc.tensor.matmul(
                        ps[:, mc * 512:(mc + 1) * 512],
                        lhsT=og_sb[:, hc, qc * 128:(qc + 1) * 128],
                        rhs=wf_sb[:, hc, mc * 512:(mc + 1) * 512],
                        start=(hc == 0), stop=(hc == NHC - 1))
            rs = work.tile([128, D_MODEL], F32, tag="rs", name="rs")
            nc.sync.dma_start(out=rs, in_=resid[qc * 128:(qc + 1) * 128, :])
            x = work.tile([128, D_MODEL], F32, tag="x", name="x")
            nc.vector.tensor_add(out=x, in0=ps, in1=rs)

            stats = stp.tile([128, 2, 6], F32, tag="stats", name="stats")
            for sg in range(2):
                nc.vector.bn_stats(out=stats[:, sg, :],
                                   in_=x[:, sg * 512:(sg + 1) * 512])
            mv = stp.tile([128, 2], F32, tag="mv", name="mv")
            nc.vector.bn_aggr(out=mv, in_=stats)
            rstd = stp.tile([128, 1], F32, tag="rstd", name="rstd")
            nc.scalar.activation(out=rstd, in_=mv[:, 1:2], func=AF.Sqrt,
                                 bias=eps_sb)
            nc.vector.reciprocal(out=rstd, in_=rstd)

            xn = work.tile([128, D_MODEL], F32, tag="xn", name="xn")
            nc.vector.tensor_scalar(out=xn, in0=x, scalar1=mv[:, 0:1],
                                    scalar2=rstd,
                                    op0=mybir.AluOpType.subtract,
                                    op1=mybir.AluOpType.mult)
            xg = work.tile([128, D_MODEL], F32, tag="xg", name="xg")
            nc.vector.tensor_mul(out=xg, in0=xn, in1=gr_sb)
            xo = work.tile([128, D_MODEL], F32, tag="xo", name="xo")
            nc.vector.tensor_add(out=xo, in0=xg, in1=br_sb)
            nc.sync.dma_start(out=outp[qc * 128:(qc + 1) * 128, :], in_=xo)

    nc.compile()
    return nc


# --------------------------------------------------------------------------
# Host wrapper
# --------------------------------------------------------------------------

_CACHE = {}


def _kernels():
    if "a" not in _CACHE:
        _CACHE["a"] = build_kernel_a()
        _CACHE["b"] = build_kernel_b()
    return _CACHE["a"], _CACHE["b"]


def kernel(q, k, v, Wq, Wk, Wv, Wfc, gamma, beta, mask):
    bf16 = ml_dtypes.bfloat16
    f8 = ml_dtypes.float8_e5m2
    q = np.asarray(q, np.float32)
    k = np.asarray(k, np.float32)
    v = np.asarray(v, np.float32)
    mask = np.asarray(mask)
    nca, ncb = _kernels()

    temp = float(np.float32(D_K) ** 0.5)
    wq_s = (np.asarray(Wq, np.float32) / temp).astype(bf16)
    wk_s = np.asarray(Wk, np.float32).astype(bf16)
    wv_s = np.asarray(Wv, np.float32).astype(bf16)
    ident = np.eye(128, dtype=np.float32).astype(f8)

    qTb, kTb, vTb, mbTb = [], [], [], []
    for b in range(B):
        qTb.append(np.ascontiguousarray(q[b].astype(bf16).T))
        kTb.append(np.ascontiguousarray(k[b].astype(bf16).T))
        vTb.append(np.ascontiguousarray(v[b].astype(bf16).T))
        mbTb.append(np.ascontiguousarray(
            np.where(mask[b] == 0, np.float32(MASK_NEG), np.float32(0.0)).T
        ).astype(f8))

    in_maps_a = []
    for core in range(N_CORES):
        b, g = core // HG, core % HG
        cs = slice(g * HG * D_K, (g + 1) * HG * D_K)
        in_maps_a.append({
            "qT": qTb[b], "kT": kTb[b], "vT": vTb[b], "maskbT": mbTb[b],
            "wq": np.ascontiguousarray(wq_s[:, cs]),
            "wk": np.ascontiguousarray(wk_s[:, cs]),
            "wv": np.ascontiguousarray(wv_s[:, cs]),
            "ident": ident,
        })
    res_a = run_bass_kernel_spmd(nca, in_maps_a, list(range(N_CORES)),
                                 trace=TRACE)
    if TRACE and res_a.exec_time_ns is not None:
        _CACHE["exec_a"] = res_a.exec_time_ns

    # assemble attn output; build launch-B inputs
    attn = np.empty((B, N_HEAD, S, S), np.float32)
    for core in range(N_CORES):
        b, g = core // HG, core % HG
        at = res_a.results[core]["attnT"]
        for jh in range(HG):
            attn[b, HG * g + jh] = at[jh].T

    wfc_s = np.asarray(Wfc, np.float32).astype(bf16)
    gamma_rep = np.tile(np.asarray(gamma, np.float32)[None, :], (128, 1))
    beta_rep = np.tile(np.asarray(beta, np.float32)[None, :], (128, 1))
    SL = S // 4
    in_maps_b = []
    for core in range(N_CORES):
        b, g = core // HG, core % HG
        rows = slice(g * SL, (g + 1) * SL)
        og = np.empty((N_HEAD * D_V, SL), bf16)
        for g2 in range(HG):
            ot = res_a.results[b * HG + g2]["outT"]  # [HG, D_V, S]
            for jh in range(HG):
                h = HG * g2 + jh
                og[h * D_V:(h + 1) * D_V, :] = ot[jh][:, rows]
        in_maps_b.append({
            "outTg": og,
            "wfc": wfc_s,
            "resid": np.ascontiguousarray(q[b, rows, :]),
            "gamma_rep": gamma_rep,
            "beta_rep": beta_rep,
        })
    res_b = run_bass_kernel_spmd(ncb, in_maps_b, list(range(N_CORES)),
                                 trace=TRACE)
    if TRACE and res_b.exec_time_ns is not None:
        _CACHE["exec_b"] = res_b.exec_time_ns

    out = np.empty((B, S, D_MODEL), np.float32)
    for core in range(N_CORES):
        b, g = core // HG, core % HG
        out[b, g * SL:(g + 1) * SL, :] = res_b.results[core]["outp"]
    return out, attn


if __name__ == "__main__":
    rng = np.random.default_rng(0)
    ins = {
        "q": rng.standard_normal((B, S, D_MODEL), np.float32),
        "k": rng.standard_normal((B, S, D_MODEL), np.float32),
        "v": rng.standard_normal((B, S, D_MODEL), np.float32),
        "Wq": (rng.standard_normal((D_MODEL, N_HEAD * D_K), np.float32) * 0.02),
        "Wk": (rng.standard_normal((D_MODEL, N_HEAD * D_K), np.float32) * 0.02),
        "Wv": (rng.standard_normal((D_MODEL, N_HEAD * D_V), np.float32) * 0.02),
        "Wfc": (rng.standard_normal((N_HEAD * D_V, D_MODEL), np.float32) * 0.02),
        "gamma": np.ones(D_MODEL, np.float32),
        "beta": np.zeros(D_MODEL, np.float32),
        "mask": rng.integers(0, 2, (B, S, S)).astype(np.int32),
    }
    out, attn = kernel(**ins)
    print("ok", out.shape, attn.shape)
